# revision 1
# baseline (speedup 1.0000x reference)
"""Multi-head self-attention (L=2048, N=2, E=1024, H=16, causal) on 8 TRN2
NeuronCores.

Strategy: tensor-parallel over heads. Each core c owns heads {2c, 2c+1}
(E-dims [128c, 128c+128)):
  - computes Q/K/V projections for its 128 dims over all 4096 tokens
    (tokens de-interleaved host-side to n-major order),
  - runs causal attention for its 4 (batch, head) pairs with a fused
    ones-column denominator trick,
  - AllToAll redistributes ctx^T so core c holds all 1024 E-dims for its
    512-token slice,
  - each core computes its [512, 1024] slice of out_proj; host reassembles.

Matmuls run in float32r (TF32-like, full-rate on the PE); accumulation fp32.
"""

import sys

if "/opt/trn_rl_repo" not in sys.path:
    sys.path.insert(0, "/opt/trn_rl_repo")

import numpy as np

import concourse.bacc as bacc
import concourse.tile as tile
import concourse.mybir as mybir
from concourse.bass_utils import run_bass_kernel_spmd

NCORES = 8
L, N, E = 2048, 2, 1024
H, DH = 16, 64
G = L * N  # 4096 global tokens
TPC = G // NCORES  # 512 tokens per core
SCALE = DH ** -0.5
NEG = -10000.0

f32 = mybir.dt.float32
f32r = mybir.dt.float32r
Exp = mybir.ActivationFunctionType.Exp

_STATE = None  # (nc, core_ids)


def _build_program():
    nc = bacc.Bacc("TRN2", target_bir_lowering=False, debug=False,
                   num_devices=NCORES)

    qT_in = nc.declare_dram_parameter("qT", [E, G], f32r, isOutput=False)
    wq_in = nc.declare_dram_parameter("wq", [E, 128], f32r, isOutput=False)
    wk_in = nc.declare_dram_parameter("wk", [E, 128], f32r, isOutput=False)
    wv_in = nc.declare_dram_parameter("wv", [E, 128], f32r, isOutput=False)
    wo_in = nc.declare_dram_parameter("wo", [E, E], f32r, isOutput=False)
    bq_in = nc.declare_dram_parameter("bq", [128, 1], f32, isOutput=False)
    bk_in = nc.declare_dram_parameter("bk", [128, 1], f32, isOutput=False)
    bv_in = nc.declare_dram_parameter("bv", [128, 1], f32, isOutput=False)
    bo_in = nc.declare_dram_parameter("bo", [128, E], f32, isOutput=False)
    mask_in = nc.declare_dram_parameter("mask", [4, 128, 512], f32r,
                                        isOutput=False)
    ident_in = nc.declare_dram_parameter("ident", [128, 128], f32r,
                                         isOutput=False)
    ones_in = nc.declare_dram_parameter("ones", [128, 64], f32r,
                                        isOutput=False)
    y_out = nc.declare_dram_parameter("y", [TPC, E], f32, isOutput=True)

    from contextlib import ExitStack

    with tile.TileContext(nc) as tc, ExitStack() as stk:
        const = stk.enter_context(tc.tile_pool(name="const", bufs=1))
        qk = stk.enter_context(tc.tile_pool(name="qk", bufs=1))
        vpp = stk.enter_context(tc.tile_pool(name="vpp", bufs=1))

        wq_t = [const.tile([128, 128], f32r, name=f"wq{e}") for e in range(8)]
        wk_t = [const.tile([128, 128], f32r, name=f"wk{e}") for e in range(8)]
        wv_t = [const.tile([128, 128], f32r, name=f"wv{e}") for e in range(8)]
        mask_t = [const.tile([128, 512], f32r, name=f"mask{j}") for j in range(4)]
        ident_t = const.tile([128, 128], f32r, name="ident")
        ones_t = const.tile([128, 64], f32r, name="ones")
        bq_t = const.tile([128, 1], f32, name="bq")
        bk_t = const.tile([128, 1], f32, name="bk")
        bv_t = const.tile([128, 1], f32, name="bv")
        bo_t = const.tile([128, E], f32, name="bo")
        for e in range(8):
            nc.sync.dma_start(out=wq_t[e][:], in_=wq_in[128 * e : 128 * e + 128, :])
            nc.sync.dma_start(out=wk_t[e][:], in_=wk_in[128 * e : 128 * e + 128, :])
            nc.sync.dma_start(out=wv_t[e][:], in_=wv_in[128 * e : 128 * e + 128, :])
        for j in range(4):
            nc.sync.dma_start(out=mask_t[j][:], in_=mask_in[j, :, :])
        nc.sync.dma_start(out=ident_t[:], in_=ident_in[:])
        nc.sync.dma_start(out=ones_t[:], in_=ones_in[:])
        nc.sync.dma_start(out=bq_t[:], in_=bq_in[:])
        nc.sync.dma_start(out=bk_t[:], in_=bk_in[:])
        nc.sync.dma_start(out=bv_t[:], in_=bv_in[:])
        nc.sync.dma_start(out=bo_t[:], in_=bo_in[:])

        QT = qk.tile([128, G], f32r, name="QT")
        KT = qk.tile([128, G], f32r, name="KT")
        ctxT = [qk.tile([64, G], f32r, name=f"ctxT{h}") for h in range(2)]
        # V' tiles: [128(k), 64 data + 1 ones] per (n, head, k-chunk-of-128)
        vp = [[[vpp.tile([128, 65], f32r, name=f"vp{n}_{h}_{kc}")
                for kc in range(16)] for h in range(2)] for n in range(2)]
        for n in range(2):
            for h in range(2):
                for kc in range(16):
                    nc.vector.tensor_copy(vp[n][h][kc][:, 64:65],
                                          ones_t[:, 0:1])

        # ---- Phase 1: projections + V transpose ----
        with (
            tc.tile_pool(name="qs", bufs=2) as qs,
            tc.tile_pool(name="vtmp", bufs=2) as vtmp,
            tc.tile_pool(name="psA", bufs=3, space="PSUM") as psA,
            tc.tile_pool(name="psT", bufs=2, space="PSUM") as psT,
        ):
            for tg in range(8):
                n, lc4 = divmod(tg, 4)
                col0 = 512 * tg
                qts = [qs.tile([128, 512], f32r, name=f"qts{e}", tag=f"e{e}")
                       for e in range(8)]
                for e in range(8):
                    nc.sync.dma_start(
                        out=qts[e][:],
                        in_=qT_in[128 * e : 128 * e + 128, col0 : col0 + 512],
                    )
                for which in range(3):
                    ps = psA.tile([128, 512], f32, tag="proj", name=f"ps{tg}_{which}")
                    w_t = (wq_t, wk_t, wv_t)[which]
                    for e in range(8):
                        nc.tensor.matmul(ps[:], w_t[e][:], qts[e][:],
                                         start=(e == 0), stop=(e == 7))
                    if which == 0:
                        nc.vector.tensor_scalar_add(
                            QT[:, col0 : col0 + 512], ps[:], bq_t[:])
                    elif which == 1:
                        nc.vector.tensor_scalar_add(
                            KT[:, col0 : col0 + 512], ps[:], bk_t[:])
                    else:
                        vt = vtmp.tile([128, 512], f32r, tag="vt", name=f"vt{tg}")
                        nc.vector.tensor_scalar_add(vt[:], ps[:], bv_t[:])
                        for b in range(4):
                            pt = psT.tile([128, 128], f32r, tag="tp",
                                          name=f"pt{tg}_{b}")
                            nc.tensor.transpose(
                                pt[:], vt[:, 128 * b : 128 * b + 128], ident_t[:])
                            kcg = 4 * lc4 + b
                            for hr in range(2):
                                nc.vector.tensor_copy(
                                    vp[n][hr][kcg][:, 0:64],
                                    pt[:, 64 * hr : 64 * hr + 64])

        # wo loads: emitted here so they overlap the attention phase
        wop = stk.enter_context(tc.tile_pool(name="wop", bufs=1))
        wo_t = [wop.tile([128, E], f32r, name=f"wo{d}") for d in range(8)]
        for d in range(8):
            nc.sync.dma_start(out=wo_t[d][:], in_=wo_in[128 * d : 128 * d + 128, :])

        # ---- Phase 2: causal attention ----
        with (
            tc.tile_pool(name="pp", bufs=3) as pp,
            tc.tile_pool(name="misc", bufs=2) as misc,
            tc.tile_pool(name="psS", bufs=1, space="PSUM") as psS,
            tc.tile_pool(name="psC", bufs=1, space="PSUM") as psC,
        ):
            for n in range(2):
                for qc in range(4):
                    nk = 4 * qc + 4
                    q0 = 2048 * n + 512 * qc
                    s_t = [psS.tile([128, 1024], f32, tag=f"s{hr}",
                                    name=f"s{n}_{qc}_{hr}") for hr in range(2)]
                    c_t = [psC.tile([65, 512], f32, tag=f"c{hr}",
                                    name=f"c{n}_{qc}_{hr}") for hr in range(2)]
                    for kq in range(nk // 2):
                        for half in range(2):
                            kc = 2 * kq + half
                            j = kc - 4 * qc
                            k0 = 2048 * n + 128 * kc
                            for hr in range(2):
                                r0 = 64 * hr
                                nc.tensor.matmul(
                                    s_t[hr][:, 512 * half : 512 * half + 512],
                                    KT[r0 : r0 + 64, k0 : k0 + 128],
                                    QT[r0 : r0 + 64, q0 : q0 + 512],
                                    start=True, stop=(j < 0),
                                    tile_position=(r0, 0),
                                )
                                if j >= 0:
                                    nc.tensor.matmul(
                                        s_t[hr][:, 512 * half : 512 * half + 512],
                                        ident_t[:], mask_t[j][:],
                                        start=False, stop=True,
                                    )
                        for hr in range(2):
                            p = pp.tile([128, 1024], f32r, tag=f"p{hr}",
                                        name=f"p{n}_{qc}_{kq}_{hr}")
                            nc.scalar.activation(p[:], s_t[hr][:], Exp)
                            for half in range(2):
                                kc = 2 * kq + half
                                nc.tensor.matmul(
                                    c_t[hr][:], vp[n][hr][kc][:],
                                    p[:, 512 * half : 512 * half + 512],
                                    start=(kc == 0), stop=(kc == nk - 1),
                                )
                    for hr in range(2):
                        recip = misc.tile([65, 512], f32r, tag="recip",
                                          name=f"re{n}_{qc}_{hr}")
                        with nc.allow_low_precision(reason="f32r recip for PE bcast"):
                            nc.vector.reciprocal(recip[64:65, :],
                                                 c_t[hr][64:65, :])
                        bc = psC.tile([64, 512], f32, tag="bc",
                                      name=f"bc{n}_{qc}_{hr}")
                        nc.tensor.matmul(bc[:], ones_t[64:65, 0:64],
                                         recip[64:65, :], start=True, stop=True)
                        rbc = misc.tile([64, 512], f32, tag="rbc",
                                        name=f"rb{n}_{qc}_{hr}")
                        nc.any.tensor_copy(rbc[:], bc[:])
                        nc.vector.tensor_mul(
                            ctxT[hr][:, q0 : q0 + 512], c_t[hr][0:64, :], rbc[:])

        # ---- Phase 3: AllToAll + output projection ----
        with (
            tc.tile_pool(name="a2asb", bufs=1) as a2asb,
            tc.tile_pool(name="osb", bufs=2) as osb,
            tc.tile_pool(name="psO", bufs=2, space="PSUM") as psO,
            tc.tile_pool(name="dram", bufs=1, space="DRAM") as dram,
        ):
            a2a_in = dram.tile([NCORES, 128, 512], f32r, name="a2a_in")
            a2a_out = dram.tile([NCORES, 128, 512], f32r, name="a2a_out")
            for jj in range(NCORES):
                nc.sync.dma_start(out=a2a_in[jj, 0:64, :],
                                  in_=ctxT[0][:, 512 * jj : 512 * jj + 512])
                nc.sync.dma_start(out=a2a_in[jj, 64:128, :],
                                  in_=ctxT[1][:, 512 * jj : 512 * jj + 512])
            nc.gpsimd.collective_compute(
                "AllToAll", mybir.AluOpType.bypass,
                replica_groups=[list(range(NCORES))],
                ins=[a2a_in.opt()], outs=[a2a_out.opt()],
            )
            a2a_t = [a2asb.tile([128, 512], f32r, name=f"a2a{d}")
                     for d in range(8)]
            for d in range(8):
                nc.sync.dma_start(out=a2a_t[d][:], in_=a2a_out[d, :, :])
            for tsub in range(4):
                ob = osb.tile([128, E], f32, tag="ob", name=f"ob{tsub}")
                for oc in range(2):
                    po = psO.tile([128, 512], f32, tag="po",
                                  name=f"po{tsub}_{oc}")
                    for d in range(8):
                        nc.tensor.matmul(
                            po[:],
                            a2a_t[d][:, 128 * tsub : 128 * tsub + 128],
                            wo_t[d][:, 512 * oc : 512 * oc + 512],
                            start=(d == 0), stop=(d == 7),
                        )
                    nc.vector.tensor_add(ob[:, 512 * oc : 512 * oc + 512],
                                         po[:], bo_t[:, 512 * oc : 512 * oc + 512])
                nc.sync.dma_start(out=y_out[128 * tsub : 128 * tsub + 128, :],
                                  in_=ob[:])

    nc.finalize()
    return nc


def _get_state():
    global _STATE
    if _STATE is None:
        nc = _build_program()
        _STATE = (nc, list(range(NCORES)))
    return _STATE


def _host_prep(inputs):
    query = np.ascontiguousarray(np.asarray(inputs["query"], np.float32))
    q_proj = np.asarray(inputs["q_proj"], np.float32)
    q_bias = np.asarray(inputs["q_bias"], np.float32)
    k_proj = np.asarray(inputs["k_proj"], np.float32)
    k_bias = np.asarray(inputs["k_bias"], np.float32)
    v_proj = np.asarray(inputs["v_proj"], np.float32)
    v_bias = np.asarray(inputs["v_bias"], np.float32)
    out_proj = np.asarray(inputs["out_proj"], np.float32)
    out_bias = np.asarray(inputs["out_bias"], np.float32)

    # [L, N, E] -> [E, N*L] n-major token order
    qT = np.ascontiguousarray(query.transpose(2, 1, 0).reshape(E, G))
    wo = np.ascontiguousarray(out_proj.T)
    bo = np.ascontiguousarray(np.tile(out_bias[None, :], (128, 1)))
    kr = np.arange(128, dtype=np.int64)[:, None]
    qr = np.arange(512, dtype=np.int64)[None, :]
    mask = np.zeros((4, 128, 512), np.float32)
    for j in range(4):
        mask[j] = np.where(kr > qr - 128 * j, NEG, 0.0)
    ident = np.eye(128, dtype=np.float32)
    ones = np.ones((128, 64), np.float32)

    in_maps = []
    for c in range(NCORES):
        dlo = 128 * c
        sl = slice(dlo, dlo + 128)
        in_maps.append({
            "qT": qT,
            "wq": np.ascontiguousarray((q_proj[sl] * SCALE).T),
            "wk": np.ascontiguousarray(k_proj[sl].T),
            "wv": np.ascontiguousarray(v_proj[sl].T),
            "wo": wo,
            "bq": np.ascontiguousarray((q_bias[sl] * SCALE)[:, None]),
            "bk": np.ascontiguousarray(k_bias[sl][:, None]),
            "bv": np.ascontiguousarray(v_bias[sl][:, None]),
            "bo": bo,
            "mask": mask,
            "ident": ident,
            "ones": ones,
        })
    return in_maps


def kernel(**inputs) -> np.ndarray:
    nc, core_ids = _get_state()
    in_maps = _host_prep(inputs)
    res = run_bass_kernel_spmd(nc, in_maps, core_ids)
    y = np.concatenate([res.results[c]["y"] for c in range(NCORES)], axis=0)
    # [G, E] n-major -> [L, N, E]
    out = y.reshape(N, L, E).transpose(1, 0, 2)
    return np.ascontiguousarray(out)


# revision 4
# speedup vs baseline: 1.4095x; 1.4095x over previous
"""Multi-head self-attention (L=2048, N=2, E=1024, H=16, causal) on 8 TRN2
NeuronCores.

Strategy: tensor-parallel over heads. Each core c owns heads {2c, 2c+1}
(E-dims [128c, 128c+128)):
  - computes Q/K/V projections for its 128 dims over all 4096 tokens
    (tokens de-interleaved host-side to n-major order),
  - runs causal attention for its 4 (batch, head) pairs with a fused
    ones-column denominator trick,
  - AllToAll redistributes ctx^T so core c holds all 1024 E-dims for its
    512-token slice,
  - each core computes its [512, 1024] slice of out_proj; host reassembles.

Matmuls run in float32r (TF32-like, full-rate on the PE); accumulation fp32.
"""

import sys

if "/opt/trn_rl_repo" not in sys.path:
    sys.path.insert(0, "/opt/trn_rl_repo")

import numpy as np

import concourse.bacc as bacc
import concourse.tile as tile
import concourse.mybir as mybir

NCORES = 8
L, N, E = 2048, 2, 1024
H, DH = 16, 64
G = L * N  # 4096 global tokens
TPC = G // NCORES  # 512 tokens per core
SCALE = DH ** -0.5
NEG = -10000.0

f32 = mybir.dt.float32
f32r = mybir.dt.float32r
Exp = mybir.ActivationFunctionType.Exp

_STATE = None  # (nc, core_ids)


def _build_program():
    nc = bacc.Bacc("TRN2", target_bir_lowering=False, debug=False,
                   num_devices=NCORES)

    qT_in = nc.declare_dram_parameter("qT", [E, G], f32r, isOutput=False)
    wq_in = nc.declare_dram_parameter("wq", [E, 128], f32r, isOutput=False)
    wk_in = nc.declare_dram_parameter("wk", [E, 128], f32r, isOutput=False)
    wv_in = nc.declare_dram_parameter("wv", [E, 128], f32r, isOutput=False)
    wo_in = nc.declare_dram_parameter("wo", [E, E], f32r, isOutput=False)
    bq_in = nc.declare_dram_parameter("bq", [128, 1], f32, isOutput=False)
    bk_in = nc.declare_dram_parameter("bk", [128, 1], f32, isOutput=False)
    bv_in = nc.declare_dram_parameter("bv", [128, 1], f32, isOutput=False)
    bo_in = nc.declare_dram_parameter("bo", [128, E], f32, isOutput=False)
    mask_in = nc.declare_dram_parameter("mask", [4, 128, 512], f32r,
                                        isOutput=False)
    ident_in = nc.declare_dram_parameter("ident", [128, 128], f32r,
                                         isOutput=False)
    ones_in = nc.declare_dram_parameter("ones", [128, 64], f32r,
                                        isOutput=False)
    y_out = nc.declare_dram_parameter("y", [TPC, E], f32, isOutput=True)

    from contextlib import ExitStack

    with tile.TileContext(nc) as tc, ExitStack() as stk:
        const = stk.enter_context(tc.tile_pool(name="const", bufs=1))
        qk = stk.enter_context(tc.tile_pool(name="qk", bufs=1))
        vpp = stk.enter_context(tc.tile_pool(name="vpp", bufs=1))

        wq_t = [const.tile([128, 128], f32r, name=f"wq{e}") for e in range(8)]
        wk_t = [const.tile([128, 128], f32r, name=f"wk{e}") for e in range(8)]
        wv_t = [const.tile([128, 128], f32r, name=f"wv{e}") for e in range(8)]
        mask_t = [const.tile([128, 512], f32r, name=f"mask{j}") for j in range(4)]
        ident_t = const.tile([128, 128], f32r, name="ident")
        ones_t = const.tile([128, 64], f32r, name="ones")
        bq_t = const.tile([128, 1], f32, name="bq")
        bk_t = const.tile([128, 1], f32, name="bk")
        bv_t = const.tile([128, 1], f32, name="bv")
        bo_t = const.tile([128, E], f32, name="bo")
        for e in range(8):
            nc.sync.dma_start(out=wq_t[e][:], in_=wq_in[128 * e : 128 * e + 128, :])
            nc.sync.dma_start(out=wk_t[e][:], in_=wk_in[128 * e : 128 * e + 128, :])
            nc.sync.dma_start(out=wv_t[e][:], in_=wv_in[128 * e : 128 * e + 128, :])
        for j in range(4):
            nc.sync.dma_start(out=mask_t[j][:], in_=mask_in[j, :, :])
        nc.sync.dma_start(out=ident_t[:], in_=ident_in[:])
        nc.sync.dma_start(out=ones_t[:], in_=ones_in[:])
        nc.sync.dma_start(out=bq_t[:], in_=bq_in[:])
        nc.sync.dma_start(out=bk_t[:], in_=bk_in[:])
        nc.sync.dma_start(out=bv_t[:], in_=bv_in[:])
        nc.sync.dma_start(out=bo_t[:], in_=bo_in[:])

        QT = qk.tile([128, G], f32r, name="QT")
        KT = qk.tile([128, G], f32r, name="KT")
        ctxT = [qk.tile([64, G], f32r, name=f"ctxT{h}") for h in range(2)]
        # V' tiles: [128(k), 64 data + 1 ones] per (n, head, k-chunk-of-128)
        vp = [[[vpp.tile([128, 65], f32r, name=f"vp{n}_{h}_{kc}")
                for kc in range(16)] for h in range(2)] for n in range(2)]
        for n in range(2):
            for h in range(2):
                for kc in range(16):
                    nc.vector.tensor_copy(vp[n][h][kc][:, 64:65],
                                          ones_t[:, 0:1])

        # ---- Phase 1: projections + V transpose ----
        with (
            tc.tile_pool(name="qs", bufs=2) as qs,
            tc.tile_pool(name="vtmp", bufs=2) as vtmp,
            tc.tile_pool(name="psA", bufs=3, space="PSUM") as psA,
            tc.tile_pool(name="psT", bufs=2, space="PSUM") as psT,
        ):
            for tg in range(8):
                n, lc4 = divmod(tg, 4)
                col0 = 512 * tg
                qts = [qs.tile([128, 512], f32r, name=f"qts{e}", tag=f"e{e}")
                       for e in range(8)]
                for e in range(8):
                    nc.sync.dma_start(
                        out=qts[e][:],
                        in_=qT_in[128 * e : 128 * e + 128, col0 : col0 + 512],
                    )
                for which in range(3):
                    ps = psA.tile([128, 512], f32, tag="proj", name=f"ps{tg}_{which}")
                    w_t = (wq_t, wk_t, wv_t)[which]
                    for e in range(8):
                        nc.tensor.matmul(ps[:], w_t[e][:], qts[e][:],
                                         start=(e == 0), stop=(e == 7))
                    if which == 0:
                        nc.vector.tensor_scalar_add(
                            QT[:, col0 : col0 + 512], ps[:], bq_t[:])
                    elif which == 1:
                        nc.vector.tensor_scalar_add(
                            KT[:, col0 : col0 + 512], ps[:], bk_t[:])
                    else:
                        vt = vtmp.tile([128, 512], f32r, tag="vt", name=f"vt{tg}")
                        nc.vector.tensor_scalar_add(vt[:], ps[:], bv_t[:])
                        for b in range(4):
                            pt = psT.tile([128, 128], f32r, tag="tp",
                                          name=f"pt{tg}_{b}")
                            nc.tensor.transpose(
                                pt[:], vt[:, 128 * b : 128 * b + 128], ident_t[:])
                            kcg = 4 * lc4 + b
                            for hr in range(2):
                                nc.vector.tensor_copy(
                                    vp[n][hr][kcg][:, 0:64],
                                    pt[:, 64 * hr : 64 * hr + 64])

        # wo loads: emitted here so they overlap the attention phase
        wop = stk.enter_context(tc.tile_pool(name="wop", bufs=1))
        wo_t = [wop.tile([128, E], f32r, name=f"wo{d}") for d in range(8)]
        for d in range(8):
            nc.sync.dma_start(out=wo_t[d][:], in_=wo_in[128 * d : 128 * d + 128, :])

        # ---- Phase 2: causal attention ----
        with (
            tc.tile_pool(name="pp", bufs=3) as pp,
            tc.tile_pool(name="misc", bufs=2) as misc,
            tc.tile_pool(name="psS", bufs=1, space="PSUM") as psS,
            tc.tile_pool(name="psC", bufs=1, space="PSUM") as psC,
        ):
            for n in range(2):
                for qc in range(4):
                    nk = 4 * qc + 4
                    q0 = 2048 * n + 512 * qc
                    s_t = [psS.tile([128, 1024], f32, tag=f"s{hr}",
                                    name=f"s{n}_{qc}_{hr}") for hr in range(2)]
                    c_t = [psC.tile([65, 512], f32, tag=f"c{hr}",
                                    name=f"c{n}_{qc}_{hr}") for hr in range(2)]
                    for kq in range(nk // 2):
                        for half in range(2):
                            kc = 2 * kq + half
                            j = kc - 4 * qc
                            k0 = 2048 * n + 128 * kc
                            for hr in range(2):
                                r0 = 64 * hr
                                nc.tensor.matmul(
                                    s_t[hr][:, 512 * half : 512 * half + 512],
                                    KT[r0 : r0 + 64, k0 : k0 + 128],
                                    QT[r0 : r0 + 64, q0 : q0 + 512],
                                    start=True, stop=(j < 0),
                                    tile_position=(r0, 0),
                                )
                                if j >= 0:
                                    nc.tensor.matmul(
                                        s_t[hr][:, 512 * half : 512 * half + 512],
                                        ident_t[:], mask_t[j][:],
                                        start=False, stop=True,
                                    )
                        for hr in range(2):
                            p = pp.tile([128, 1024], f32r, tag=f"p{hr}",
                                        name=f"p{n}_{qc}_{kq}_{hr}")
                            nc.scalar.activation(p[:], s_t[hr][:], Exp)
                            for half in range(2):
                                kc = 2 * kq + half
                                nc.tensor.matmul(
                                    c_t[hr][:], vp[n][hr][kc][:],
                                    p[:, 512 * half : 512 * half + 512],
                                    start=(kc == 0), stop=(kc == nk - 1),
                                )
                    for hr in range(2):
                        recip = misc.tile([65, 512], f32r, tag="recip",
                                          name=f"re{n}_{qc}_{hr}")
                        with nc.allow_low_precision(reason="f32r recip for PE bcast"):
                            nc.vector.reciprocal(recip[64:65, :],
                                                 c_t[hr][64:65, :])
                        bc = psC.tile([64, 512], f32, tag="bc",
                                      name=f"bc{n}_{qc}_{hr}")
                        nc.tensor.matmul(bc[:], ones_t[64:65, 0:64],
                                         recip[64:65, :], start=True, stop=True)
                        rbc = misc.tile([64, 512], f32, tag="rbc",
                                        name=f"rb{n}_{qc}_{hr}")
                        nc.any.tensor_copy(rbc[:], bc[:])
                        nc.vector.tensor_mul(
                            ctxT[hr][:, q0 : q0 + 512], c_t[hr][0:64, :], rbc[:])

        # ---- Phase 3: AllToAll + output projection ----
        with (
            tc.tile_pool(name="a2asb", bufs=1) as a2asb,
            tc.tile_pool(name="osb", bufs=2) as osb,
            tc.tile_pool(name="psO", bufs=2, space="PSUM") as psO,
            tc.tile_pool(name="dram", bufs=1, space="DRAM") as dram,
        ):
            a2a_in = dram.tile([NCORES, 128, 512], f32r, name="a2a_in")
            a2a_out = dram.tile([NCORES, 128, 512], f32r, name="a2a_out")
            for jj in range(NCORES):
                nc.sync.dma_start(out=a2a_in[jj, 0:64, :],
                                  in_=ctxT[0][:, 512 * jj : 512 * jj + 512])
                nc.sync.dma_start(out=a2a_in[jj, 64:128, :],
                                  in_=ctxT[1][:, 512 * jj : 512 * jj + 512])
            nc.gpsimd.collective_compute(
                "AllToAll", mybir.AluOpType.bypass,
                replica_groups=[list(range(NCORES))],
                ins=[a2a_in.opt()], outs=[a2a_out.opt()],
            )
            a2a_t = [a2asb.tile([128, 512], f32r, name=f"a2a{d}")
                     for d in range(8)]
            for d in range(8):
                nc.sync.dma_start(out=a2a_t[d][:], in_=a2a_out[d, :, :])
            for tsub in range(4):
                ob = osb.tile([128, E], f32, tag="ob", name=f"ob{tsub}")
                for oc in range(2):
                    po = psO.tile([128, 512], f32, tag="po",
                                  name=f"po{tsub}_{oc}")
                    for d in range(8):
                        nc.tensor.matmul(
                            po[:],
                            a2a_t[d][:, 128 * tsub : 128 * tsub + 128],
                            wo_t[d][:, 512 * oc : 512 * oc + 512],
                            start=(d == 0), stop=(d == 7),
                        )
                    nc.vector.tensor_add(ob[:, 512 * oc : 512 * oc + 512],
                                         po[:], bo_t[:, 512 * oc : 512 * oc + 512])
                nc.sync.dma_start(out=y_out[128 * tsub : 128 * tsub + 128, :],
                                  in_=ob[:])

    nc.finalize()
    return nc


# Inputs identical on every core -> replicated (shipped once), the rest are
# per-core and stacked along axis 0.
_SHARED = {"qT", "wo", "bo", "mask", "ident", "ones"}


def _get_state():
    """Build the Bass program once and return a cached jitted executor."""
    global _STATE
    if _STATE is not None:
        return _STATE

    import jax
    import jax.numpy as jnp
    from jax.sharding import Mesh, NamedSharding, PartitionSpec
    from jax.experimental.shard_map import shard_map
    import concourse.bass2jax as bass2jax

    nc = _build_program()
    bass2jax.install_neuronx_cc_hook()

    partition_name = (nc.partition_id_tensor.name
                      if nc.partition_id_tensor else None)
    in_names: list = []
    out_names: list = []
    out_avals: list = []
    for alloc in nc.m.functions[0].allocations:
        if not isinstance(alloc, mybir.MemoryLocationSet):
            continue
        name = alloc.memorylocations[0].name
        if alloc.kind == "ExternalInput":
            if name != partition_name:
                in_names.append(name)
        elif alloc.kind == "ExternalOutput":
            out_names.append(name)
            out_avals.append(jax.core.ShapedArray(
                tuple(alloc.tensor_shape), mybir.dt.np(alloc.dtype)))
    n_params = len(in_names)
    all_in_names = list(in_names) + list(out_names)
    if partition_name is not None:
        all_in_names.append(partition_name)

    def _body(*args):
        operands = list(args)
        if partition_name is not None:
            operands.append(bass2jax.partition_id_tensor())
        outs = bass2jax._bass_exec_p.bind(
            *operands,
            out_avals=tuple(out_avals),
            in_names=tuple(all_in_names),
            out_names=tuple(out_names),
            lowering_input_output_aliases=(),
            sim_require_finite=True,
            sim_require_nnan=True,
            nc=nc,
        )
        return tuple(outs)

    devices = jax.devices()[:NCORES]
    mesh = Mesh(np.asarray(devices), ("core",))
    rep = PartitionSpec()
    shd = PartitionSpec("core")
    in_specs = tuple(rep if nm in _SHARED else shd for nm in in_names) \
        + (shd,) * len(out_names)
    out_specs = (shd,) * len(out_names)
    donate = tuple(range(n_params, n_params + len(out_names)))
    fn = jax.jit(
        shard_map(_body, mesh=mesh, in_specs=in_specs, out_specs=out_specs,
                  check_rep=False),
        donate_argnums=donate, keep_unused=True,
    )

    rep_sh = NamedSharding(mesh, rep)
    shd_sh = NamedSharding(mesh, shd)
    out_shapes = [(NCORES * a.shape[0],) + tuple(a.shape[1:]) for a in out_avals]
    out_dtypes = [a.dtype for a in out_avals]

    def put(name, arr):
        return jax.device_put(arr, rep_sh if name in _SHARED else shd_sh)

    def run(in_maps):
        ops = []
        for nm in in_names:
            if nm in _SHARED:
                ops.append(put(nm, in_maps[0][nm]))
            else:
                ops.append(put(nm, np.concatenate(
                    [in_maps[c][nm] for c in range(NCORES)], axis=0)))
        zeros = [jnp.zeros(s, d, device=shd_sh)
                 for s, d in zip(out_shapes, out_dtypes)]
        outs = fn(*ops, *zeros)
        return {nm: np.asarray(o) for nm, o in zip(out_names, outs)}

    _STATE = run
    return _STATE


def _host_prep(inputs):
    query = np.ascontiguousarray(np.asarray(inputs["query"], np.float32))
    q_proj = np.asarray(inputs["q_proj"], np.float32)
    q_bias = np.asarray(inputs["q_bias"], np.float32)
    k_proj = np.asarray(inputs["k_proj"], np.float32)
    k_bias = np.asarray(inputs["k_bias"], np.float32)
    v_proj = np.asarray(inputs["v_proj"], np.float32)
    v_bias = np.asarray(inputs["v_bias"], np.float32)
    out_proj = np.asarray(inputs["out_proj"], np.float32)
    out_bias = np.asarray(inputs["out_bias"], np.float32)

    # [L, N, E] -> [E, N*L] n-major token order
    qT = np.ascontiguousarray(query.transpose(2, 1, 0).reshape(E, G))
    wo = np.ascontiguousarray(out_proj.T)
    bo = np.ascontiguousarray(np.tile(out_bias[None, :], (128, 1)))
    kr = np.arange(128, dtype=np.int64)[:, None]
    qr = np.arange(512, dtype=np.int64)[None, :]
    mask = np.zeros((4, 128, 512), np.float32)
    for j in range(4):
        mask[j] = np.where(kr > qr - 128 * j, NEG, 0.0)
    ident = np.eye(128, dtype=np.float32)
    ones = np.ones((128, 64), np.float32)

    in_maps = []
    for c in range(NCORES):
        dlo = 128 * c
        sl = slice(dlo, dlo + 128)
        in_maps.append({
            "qT": qT,
            "wq": np.ascontiguousarray((q_proj[sl] * SCALE).T),
            "wk": np.ascontiguousarray(k_proj[sl].T),
            "wv": np.ascontiguousarray(v_proj[sl].T),
            "wo": wo,
            "bq": np.ascontiguousarray((q_bias[sl] * SCALE)[:, None]),
            "bk": np.ascontiguousarray(k_bias[sl][:, None]),
            "bv": np.ascontiguousarray(v_bias[sl][:, None]),
            "bo": bo,
            "mask": mask,
            "ident": ident,
            "ones": ones,
        })
    return in_maps


def kernel(**inputs) -> np.ndarray:
    run = _get_state()
    in_maps = _host_prep(inputs)
    y = run(in_maps)["y"]  # [G, E] n-major token order
    out = y.reshape(N, L, E).transpose(1, 0, 2)
    return np.ascontiguousarray(out)
